# revision 23
# baseline (speedup 1.0000x reference)
"""CARE-GNN Trainium2 kernel (nn_CAREGNN_62199716381202), v2.

Strategy (graph/data parallel, 8 NeuronCores):
- Shard destination nodes across the 8 cores (6250 dsts each); each core owns
  the edges incident (by dst) to its shard, sorted by dst, split into
  low-src / high-src streams (int16 gather-index limit at 32768).
- Per 128-dst block and etype, edges are gathered (dma_gather, bf16 rows,
  256B descriptors) into edge-major chunks G [128 slots, 128 feat].
- Segment-MEAN via one flipped one-hot matmul per chunk:
      psum[feat, dst] += G[slot, feat]^T @ S[slot, dst]
  where S = (iota == lbl) * (1/deg)  -- the mean scale is baked into S, so
  PSUM holds the per-etype mean directly, already in transposed [feat, dst]
  layout.  The transposed layout feeds tanh / p-weighted sum / residual /
  Linear (W @ h) with no per-block PE transposes.
- bf16 everywhere data-sized (inputs quantized ~0.4%, fine for rel<2e-2).
- Both layers share identical edge streams (same graph): one set of
  idx/label/icnt metadata, two gather sources (X rows, AllGathered H rows).
- Chunk counts per (etype, 128-dst block) are exact per-block maxima over
  cores (SPMD-uniform), not global maxima: ~15% fewer gather descriptors
  and matmuls than uniform padding.
- AllGather of layer-0 H rows (bf16, 128-padded) is chunked (5 chunks,
  chunk-major DRAM layout, boundaries aligned with the int16 lo/hi split at
  row 32768) so it overlaps layer-0 compute and layer-1 lo-gathers only
  wait on lo chunks.  Layer-1 gather indices are host-remapped through the
  chunk-major layout.
"""

import sys

if "/opt/trn_rl_repo" not in sys.path:
    sys.path.insert(0, "/opt/trn_rl_repo")

import numpy as np
import ml_dtypes

BF16 = ml_dtypes.bfloat16

import concourse.bass as bass
import concourse.bacc as bacc
import concourse.mybir as mybir
import concourse.tile as tile
from concourse.bass_utils import run_bass_kernel_spmd

F32 = mybir.dt.float32
BF = mybir.dt.bfloat16
I16 = mybir.dt.int16
ADD = mybir.AluOpType.add
MULT = mybir.AluOpType.mult
ISEQ = mybir.AluOpType.is_equal
TANH = mybir.ActivationFunctionType.Tanh


class Cfg:
    def __init__(self, N=50000, E=500000, n_cores=8, split=32768, n_ag=4):
        self.ag_bounds = (0, 1536, 3072, 4096, 5120, 6250)
        self.N = N
        self.E = E
        self.D = 128
        self.HID = 64
        self.C = 2
        self.NET = 3
        self.n_cores = n_cores
        self.split = split
        self.n_ag = n_ag
        assert N % n_cores == 0
        self.ND = N // n_cores
        self.NB = (self.ND + 127) // 128

    def bs(self, b):
        return min(128, self.ND - b * 128)


def _wrap16(flat):
    w = np.ascontiguousarray(flat.reshape(-1, 16).T).astype(np.int16)
    return np.tile(w, (8, 1))


def host_prep(cfg, inputs):
    """Build per-core input maps. Returns (in_maps, CA, CB)."""
    feat = np.asarray(inputs["feat"], np.float32)
    srcs = [np.asarray(inputs[f"src{i}"]) for i in range(cfg.NET)]
    dsts = [np.asarray(inputs[f"dst{i}"]) for i in range(cfg.NET)]

    x_rows = feat.astype(BF16)                      # [N, 128] gather source
    xT = np.ascontiguousarray(feat.T)               # [128, N] f32

    # pass 1: per (etype, block) chunk-count tables (max over cores)
    percore = []
    CAt = np.ones((cfg.NET, cfg.NB), np.int64)
    CBt = np.ones((cfg.NET, cfg.NB), np.int64)
    for k in range(cfg.n_cores):
        rows = []
        for i in range(cfg.NET):
            sel = (dsts[i] >= k * cfg.ND) & (dsts[i] < (k + 1) * cfg.ND)
            dl = (dsts[i][sel] - k * cfg.ND).astype(np.int64)
            s = srcs[i][sel].astype(np.int64)
            o = np.argsort(dl, kind="stable")
            dl, s = dl[o], s[o]
            deg = np.bincount(dl, minlength=cfg.ND)
            b = dl >> 7
            half = (s >= cfg.split).astype(np.int64)
            nA = np.bincount(b[half == 0], minlength=cfg.NB)
            nB = np.bincount(b[half == 1], minlength=cfg.NB)
            CAt[i] = np.maximum(CAt[i], -(-nA // 128))
            CBt[i] = np.maximum(CBt[i], -(-nB // 128))
            rows.append((dl, s, b, half, deg))
        percore.append(rows)

    # layer-1 table row remap (chunk-major AllGather layout).
    # Chunk boundaries align global row 32768 (= local 4096) with the
    # int16 lo/hi gather split so lo gathers only wait on lo chunks.
    bounds = list(cfg.ag_bounds)
    remap = np.zeros(cfg.N, np.int64)
    nodes = np.arange(cfg.N)
    kk, rr = nodes // cfg.ND, nodes % cfg.ND
    cum = 0
    for ci in range(len(bounds) - 1):
        r0, r1 = bounds[ci], bounds[ci + 1]
        m = (rr >= r0) & (rr < r1)
        remap[m] = cum + kk[m] * (r1 - r0) + (rr[m] - r0)
        cum += cfg.n_cores * (r1 - r0)

    # layer-1 chunk tables (remapped halves)
    CAt1 = np.ones((cfg.NET, cfg.NB), np.int64)
    CBt1 = np.ones((cfg.NET, cfg.NB), np.int64)
    for k in range(cfg.n_cores):
        for i in range(cfg.NET):
            dl, s, b, half, deg = percore[k][i]
            h1 = (remap[s] >= cfg.split).astype(np.int64)
            nA = np.bincount(b[h1 == 0], minlength=cfg.NB)
            nB = np.bincount(b[h1 == 1], minlength=cfg.NB)
            CAt1[i] = np.maximum(CAt1[i], -(-nA // 128))
            CBt1[i] = np.maximum(CBt1[i], -(-nB // 128))

    def offsets(t):
        off = np.zeros(t.size + 1, np.int64)
        np.cumsum(t.reshape(-1), out=off[1:])
        return off.reshape(-1)  # flat (i*NB+b) -> chunk offset
    offA, offB = offsets(CAt), offsets(CBt)
    offA1, offB1 = offsets(CAt1), offsets(CBt1)
    GA, GB = int(offA[-1]), int(offB[-1])
    GA1, GB1 = int(offA1[-1]), int(offB1[-1])
    LA, LB = GA * 128, GB * 128
    LA1, LB1 = GA1 * 128, GB1 * 128

    Wm = np.asarray(inputs["Wm"], np.float32).astype(BF16)   # [128, 2]
    bm = np.asarray(inputs["bm"], np.float32).reshape(cfg.C, 1)
    W0 = np.asarray(inputs["W0"], np.float32).astype(BF16)   # [128, 64]
    b0 = np.asarray(inputs["b0"], np.float32).reshape(cfg.HID, 1)
    W1 = np.asarray(inputs["W1"], np.float32).astype(BF16)   # [64, 2]
    b1 = np.asarray(inputs["b1"], np.float32).reshape(cfg.C, 1)
    p0 = np.tile(np.asarray(inputs["p0"], np.float32), (128, 1))
    p1 = np.tile(np.asarray(inputs["p1"], np.float32), (128, 1))
    CH = int(max(CAt.max(), CBt.max(), CAt1.max(), CBt1.max()))
    iota3 = np.ascontiguousarray(
        np.broadcast_to(
            np.arange(128, dtype=np.float32).astype(BF16)[None, :, None],
            (128, 128, CH),
        ).reshape(128, 128 * CH)
    )
    ident = np.eye(128, dtype=np.float32).astype(BF16)

    def build_meta(k, oA, oB, nGA, nGB, use_remap):
        idxA = np.zeros(nGA * 128, np.int64)
        idxB = np.zeros(nGB * 128, np.int64)
        lblA = np.full((128, nGA), -7.0, np.float32)
        lblB = np.full((128, nGB), -7.0, np.float32)
        for i in range(cfg.NET):
            dl, s, b, half, deg = percore[k][i]
            sv = remap[s] if use_remap else s
            hv = (sv >= cfg.split).astype(np.int64)
            for half_, (idx_, lbl_, off_) in (
                (0, (idxA, lblA, oA)),
                (1, (idxB, lblB, oB)),
            ):
                m = hv == half_
                dlh, sh = dl[m], sv[m]
                bh = dlh >> 7
                cnt = np.bincount(bh, minlength=cfg.NB)
                start = np.zeros(cfg.NB + 1, np.int64)
                np.cumsum(cnt, out=start[1:])
                j = np.arange(len(dlh)) - start[bh]
                ch = off_[i * cfg.NB + bh] + (j >> 7)
                pos = ch * 128 + (j & 127)
                idx_[pos] = sh - (cfg.split if half_ else 0)
                lbl_[j & 127, ch] = dlh - bh * 128
        return idxA, idxB, lblA, lblB

    in_maps = []
    for k in range(cfg.n_cores):
        idxA, idxB, lblA, lblB = build_meta(k, offA, offB, GA, GB, False)
        idxA1, idxB1, lblA1, lblB1 = build_meta(
            k, offA1, offB1, GA1, GB1, True)
        icr = np.zeros((128, cfg.NET * cfg.NB * 128), np.float32)
        for i in range(cfg.NET):
            deg = percore[k][i][4]
            ic = 1.0 / np.maximum(deg, 1.0)
            icr[:, i * cfg.NB * 128 : i * cfg.NB * 128 + cfg.ND] = ic[None, :]
        xo = np.zeros((128, cfg.NB * 128), BF16)
        xo[:, : cfg.ND] = xT[:, k * cfg.ND : (k + 1) * cfg.ND].astype(BF16)
        in_maps.append(
            {
                "x_rows": x_rows,
                "x_ownT": xo,
                "idxA": _wrap16(idxA),
                "idxB": _wrap16(idxB),
                "lblA": lblA.astype(BF16),
                "lblB": lblB.astype(BF16),
                "icnt_rep": icr.astype(BF16),
                "idxA1": _wrap16(idxA1),
                "idxB1": _wrap16(idxB1),
                "lblA1": lblA1.astype(BF16),
                "lblB1": lblB1.astype(BF16),
                "Wm": Wm, "bm": bm, "W0": W0, "b0": b0, "W1": W1, "b1": b1,
                "p0": p0, "p1": p1, "iota3": iota3, "ident": ident,
            }
        )
    return in_maps, (tuple(CAt.reshape(-1)), tuple(CBt.reshape(-1)), tuple(CAt1.reshape(-1)), tuple(CBt1.reshape(-1)))


def build_nc(cfg, CAB, debug=False):
    CAt, CBt, CAt1, CBt1 = [np.asarray(t, np.int64) for t in CAB]
    N, ND, NB, NET, HID, C = cfg.N, cfg.ND, cfg.NB, cfg.NET, cfg.HID, cfg.C
    SPLIT = cfg.split

    def _offsets(t):
        off = np.zeros(t.size + 1, np.int64)
        np.cumsum(t, out=off[1:])
        return off

    offA, offB = _offsets(CAt), _offsets(CBt)
    offA1, offB1 = _offsets(CAt1), _offsets(CBt1)
    GA, GB = int(offA[-1]), int(offB[-1])
    GA1, GB1 = int(offA1[-1]), int(offB1[-1])
    LA, LB, LA1, LB1 = GA * 128, GB * 128, GA1 * 128, GB1 * 128
    CH = int(max(CAt.max(), CBt.max(), CAt1.max(), CBt1.max()))
    CBmax2 = int(
        max(
            (CBt[i * NB + b] + (CBt[i * NB + b + 1] if b + 1 < NB else 0))
            for i in range(NET)
            for b in range(0, NB, 2)
        )
    )
    CBmax2 = max(CBmax2, int(max(
        (CBt1[i * NB + b] + (CBt1[i * NB + b + 1] if b + 1 < NB else 0))
        for i in range(NET)
        for b in range(0, NB, 2)
    )))
    CAm = int(max(CAt.max(), CAt1.max()))
    CBm = int(max(CBt.max(), CBt1.max()))

    nc = bacc.Bacc(trn_type="TRN2", num_devices=cfg.n_cores,
                   num_swdge_queues=4)

    x_rows_d = nc.dram_tensor("x_rows", [N, 128], BF, kind="ExternalInput")
    x_ownT_d = nc.dram_tensor("x_ownT", [128, NB * 128], BF, kind="ExternalInput")
    idxA_d = nc.dram_tensor("idxA", [128, LA // 16], I16, kind="ExternalInput")
    idxB_d = nc.dram_tensor("idxB", [128, LB // 16], I16, kind="ExternalInput")
    lblA_d = nc.dram_tensor("lblA", [128, GA], BF, kind="ExternalInput")
    lblB_d = nc.dram_tensor("lblB", [128, GB], BF, kind="ExternalInput")
    icnt_d = nc.dram_tensor("icnt_rep", [128, NET * NB * 128], BF,
                            kind="ExternalInput")
    idxA1_d = nc.dram_tensor("idxA1", [128, LA1 // 16], I16, kind="ExternalInput")
    idxB1_d = nc.dram_tensor("idxB1", [128, LB1 // 16], I16, kind="ExternalInput")
    lblA1_d = nc.dram_tensor("lblA1", [128, GA1], BF, kind="ExternalInput")
    lblB1_d = nc.dram_tensor("lblB1", [128, GB1], BF, kind="ExternalInput")
    Wm_d = nc.dram_tensor("Wm", [128, C], BF, kind="ExternalInput")
    bm_d = nc.dram_tensor("bm", [C, 1], F32, kind="ExternalInput")
    W0_d = nc.dram_tensor("W0", [128, HID], BF, kind="ExternalInput")
    b0_d = nc.dram_tensor("b0", [HID, 1], F32, kind="ExternalInput")
    W1_d = nc.dram_tensor("W1", [HID, C], BF, kind="ExternalInput")
    b1_d = nc.dram_tensor("b1", [C, 1], F32, kind="ExternalInput")
    p0_d = nc.dram_tensor("p0", [128, NET], F32, kind="ExternalInput")
    p1_d = nc.dram_tensor("p1", [128, NET], F32, kind="ExternalInput")
    iota3_d = nc.dram_tensor("iota3", [128, 128 * CH], BF, kind="ExternalInput")
    ident_d = nc.dram_tensor("ident", [128, 128], BF, kind="ExternalInput")
    outT_d = nc.dram_tensor("outT", [C, ND], F32, kind="ExternalOutput")
    simT_d = nc.dram_tensor("simT", [C, ND], F32, kind="ExternalOutput")

    # AllGather chunk boundaries (after these block indices)
    ag_after = set()
    for r in cfg.ag_bounds[1:]:
        ag_after.add((r - 1) // 128)

    with tile.TileContext(nc) as tc:
        with (
            tc.tile_pool(name="const", bufs=1) as cp,
            tc.tile_pool(name="big", bufs=1) as bigp,
            tc.tile_pool(name="ix", bufs=1) as ixp,
            tc.tile_pool(name="ga", bufs=6) as gap,
            tc.tile_pool(name="gb", bufs=2) as gbp,
            tc.tile_pool(name="sgen", bufs=2) as sp,
            tc.tile_pool(name="work", bufs=2) as wp,
            tc.tile_pool(name="pse", bufs=6, space="PSUM") as pp,
            tc.tile_pool(name="po", bufs=2, space="PSUM") as pop,
            tc.tile_pool(name="dram", bufs=1, space="DRAM") as dp,
        ):
            # ---- resident constants ----
            LAm, LBm = max(LA, LA1), max(LB, LB1)
            lblA = cp.tile([128, GA], BF)
            lblB = cp.tile([128, GB], BF)
            lblA1 = cp.tile([128, GA1], BF)
            lblB1 = cp.tile([128, GB1], BF)
            icnt_rep = cp.tile([128, NET * NB * 128], BF)
            for t_, d_ in [(lblA, lblA_d), (lblB, lblB_d),
                           (lblA1, lblA1_d), (lblB1, lblB1_d),
                           (icnt_rep, icnt_d)]:
                nc.sync.dma_start(out=t_[:], in_=d_[:, :])
            x_ownT = cp.tile([128, NB * 128], BF)
            nc.sync.dma_start(out=x_ownT[:], in_=x_ownT_d[:, :])
            Wm_s = cp.tile([128, C], BF)
            bm_s = cp.tile([C, 1], F32)
            W0_s = cp.tile([128, HID], BF)
            b0_s = cp.tile([HID, 1], F32)
            W1_s = cp.tile([HID, C], BF)
            b1_s = cp.tile([C, 1], F32)
            p0_s = cp.tile([128, NET], F32)
            p1_s = cp.tile([128, NET], F32)
            iota3 = cp.tile([128, 128, CH], BF)
            ident_s = cp.tile([128, 128], BF)
            for t_, d_ in [
                (Wm_s, Wm_d), (bm_s, bm_d), (W0_s, W0_d), (b0_s, b0_d),
                (W1_s, W1_d), (b1_s, b1_d), (p0_s, p0_d), (p1_s, p1_d),
                (iota3, iota3_d.rearrange("p (j g) -> p j g", j=128)),
                (ident_s, ident_d),
            ]:
                nc.sync.dma_start(out=t_[:], in_=d_[:, :])

            hb = bigp.tile([128, NB, 128], BF)   # [h(64) | 0] feature-major
            nc.vector.memset(hb[:, :, :], 0)

            h_loc = dp.tile([ND, 128], BF)
            h_agL = dp.tile([SPLIT, 128], BF)
            h_agH = dp.tile([N - SPLIT, 128], BF)

            qctr = [0]

            def gather_batched(gtile, src, idx_tile, gc0, nchunks, tag):
                c = 0
                while c < nchunks:
                    cc = min(8, nchunks - c)
                    n = cc * 128
                    q = qctr[0] % 4
                    qctr[0] += 1
                    nc.gpsimd.dma_gather(
                        gtile[:, c : c + cc, :], src,
                        idx_tile[:, (gc0 + c) * 8 : (gc0 + c + cc) * 8],
                        n, n, 128, queue_num=q,
                    )
                    c += cc

            def make_S(S, lbl_t, gc0, CX):
                nc.vector.tensor_tensor(
                    S[:, :, 0:CX], iota3[:, :, 0:CX],
                    lbl_t[:, gc0 : gc0 + CX].unsqueeze(1).broadcast_to(
                        [128, 128, CX]),
                    ISEQ,
                )

            def layer(lnum, src_lo, src_hi, p_s, ag_row0, meta):
                """One CARE layer; returns nothing (writes outputs)."""
                ixA_d, ixB_d, lbA, lbB, oA, oB, cAt, cBt = meta
                LAx = int(oA[-1]) * 128
                LBx = int(oB[-1]) * 128
                ixA = ixp.tile([128, LAm // 16], I16, tag="ixA")
                ixB = ixp.tile([128, LBm // 16], I16, tag="ixB")
                nc.sync.dma_start(out=ixA[:, 0 : LAx // 16], in_=ixA_d[:, :])
                nc.sync.dma_start(out=ixB[:, 0 : LBx // 16], in_=ixB_d[:, :])
                NF = 128 if lnum == 0 else HID  # stationary width
                for b in range(NB):
                    # issue all 3 etypes' gathers + S-gen first
                    gAs, gBs, SAs, SBs, nAs, nBs, boffs = [], [], [], [], [], [], []
                    for i in range(NET):
                        gcA = int(oA[i * NB + b])
                        gcB = int(oB[i * NB + b])
                        nA = int(cAt[i * NB + b])
                        nB_ = int(cBt[i * NB + b])
                        gA = gap.tile([128, CH, 128], BF, tag=f"gA{lnum}")
                        gather_batched(gA, src_lo, ixA, gcA, nA, "A")
                        if b % 2 == 0:
                            nB2 = nB_ + (int(cBt[i * NB + b + 1])
                                         if b + 1 < NB else 0)
                            gBt = gbp.tile([128, CBmax2, 128], BF,
                                           tag=f"gB{lnum}{i}")
                            gather_batched(gBt, src_hi, ixB, gcB, nB2, "B")
                            gB_prev[i] = gBt
                            gB_boff[i] = nB_
                            boff = 0
                        else:
                            boff = gB_boff[i]
                        SA = sp.tile([128, 128, CAm], BF, tag=f"SA{lnum}{i}")
                        SB = sp.tile([128, 128, CBm], BF, tag=f"SB{lnum}{i}")
                        make_S(SA, lbA, gcA, nA)
                        make_S(SB, lbB, gcB, nB_)
                        gAs.append(gA); gBs.append(gB_prev[i])
                        SAs.append(SA); SBs.append(SB)
                        nAs.append(nA); nBs.append(nB_); boffs.append(boff)
                    # interleaved matmul chains (3 independent PSUM banks)
                    ps0 = pp.tile([128, 128], F32, tag="ps")
                    ps1 = pp.tile([128, 128], F32, tag="ps")
                    ps2 = pp.tile([128, 128], F32, tag="ps")
                    pss = [ps0, ps1, ps2]
                    for i in range(NET):
                        for c in range(nAs[i]):
                            nc.tensor.matmul(
                                pss[i][0:NF, :], gAs[i][:, c, 0:NF],
                                SAs[i][:, :, c],
                                start=(c == 0), stop=False,
                            )
                        for c in range(nBs[i]):
                            nc.tensor.matmul(
                                pss[i][0:NF, :],
                                gBs[i][:, boffs[i] + c, 0:NF],
                                SBs[i][:, :, c],
                                start=False, stop=(c == nBs[i] - 1),
                            )
                    ts = []
                    for i in range(NET):
                        # mean = sum * (1/deg); tanh on scalar engine
                        t = wp.tile([128, 128], F32, tag=f"t{i}")
                        nc.vector.tensor_tensor(
                            t[0:NF, :], pss[i][0:NF, :],
                            icnt_rep[0:NF, (i * NB + b) * 128
                                     : (i * NB + b + 1) * 128], MULT)
                        nc.scalar.activation(t[0:NF, :], t[0:NF, :], TANH)
                        ts.append(t)
                    # combine: acc = sum p_i * t_i + residual
                    acc = wp.tile([128, 128], F32, tag="acc")
                    nc.scalar.mul(acc[0:NF, :], ts[0][0:NF, :], p_s[0:NF, 0:1])
                    for i in range(1, NET):
                        tmp = wp.tile([128, 128], F32, tag="tmp")
                        nc.scalar.mul(
                            tmp[0:NF, :], ts[i][0:NF, :], p_s[0:NF, i : i + 1])
                        nc.vector.tensor_tensor(
                            acc[0:NF, :], acc[0:NF, :], tmp[0:NF, :], ADD)
                    if lnum == 0:
                        nc.vector.tensor_tensor(
                            acc[:], acc[:],
                            x_ownT[:, b * 128 : (b + 1) * 128], ADD)
                        h0b = wp.tile([128, 128], BF, tag="h0b")
                        nc.scalar.activation(h0b[:], acc[:], TANH)
                        # hnat = h0b @ W0 + b0  -> [64, 128]
                        po_t = pop.tile([128, 128], F32, tag="pp")
                        po = po_t[0:HID, :]
                        nc.tensor.matmul(po[:], W0_s[:], h0b[:])
                        nc.vector.tensor_scalar(
                            hb[0:HID, b, :], po[:], b0_s[:, 0:1], None, ADD)
                        # sim = tanh(x @ Wm + bm) -> [2, 128]
                        psim_t = pop.tile([128, 128], F32, tag="pp")
                        psim = psim_t[0:C, :]
                        nc.tensor.matmul(
                            psim[:], Wm_s[:],
                            x_ownT[:, b * 128 : (b + 1) * 128])
                        n = cfg.bs(b)
                        so = wp.tile([C, 128], F32, tag="so")
                        nc.scalar.activation(
                            so[:], psim[:], TANH, bias=bm_s[:, 0:1])
                        nc.sync.dma_start(
                            out=simT_d[:, b * 128 : b * 128 + n],
                            in_=so[:, 0:n])
                        # transpose hb block -> h_loc rows (via matmul w/ ident)
                        pt = pop.tile([128, 128], F32, tag="pp")
                        nc.tensor.matmul(pt[:], hb[:, b, :], ident_s[:])
                        hrow = wp.tile([128, 128], BF, tag="hrow")
                        nc.scalar.copy(hrow[:], pt[:])
                        nc.sync.dma_start(
                            out=h_loc[b * 128 : b * 128 + n, :],
                            in_=hrow[0:n, :])
                        if b in ag_after:
                            r0 = ag_row0[0]
                            r1 = min(ND, (b + 1) * 128)
                            if r1 > r0:
                                base = cfg.n_cores * r0
                                nrows = cfg.n_cores * (r1 - r0)
                                tgt = h_agL if base < SPLIT else h_agH
                                if base >= SPLIT:
                                    base -= SPLIT
                                outv = tgt[base : base + nrows, :].rearrange(
                                    "(k r) d -> k r d", k=cfg.n_cores)
                                nc.gpsimd.collective_compute(
                                    "AllGather",
                                    mybir.AluOpType.bypass,
                                    replica_groups=[list(range(cfg.n_cores))],
                                    ins=[h_loc[r0:r1, :].opt()],
                                    outs=[outv.opt()],
                                )
                            ag_row0[0] = r1
                    else:
                        nc.vector.tensor_tensor(
                            acc[0:HID, :], acc[0:HID, :], hb[0:HID, b, :], ADD)
                        h2b = wp.tile([HID, 128], BF, tag="h2b")
                        nc.scalar.activation(h2b[:], acc[0:HID, :], TANH)
                        po2_t = pop.tile([128, 128], F32, tag="pp")
                        po2 = po2_t[0:C, :]
                        nc.tensor.matmul(po2[:], W1_s[:], h2b[:])
                        n = cfg.bs(b)
                        oo = wp.tile([C, 128], F32, tag="oo")
                        nc.vector.tensor_scalar(
                            oo[:], po2[:], b1_s[:, 0:1], None, ADD)
                        nc.sync.dma_start(
                            out=outT_d[:, b * 128 : b * 128 + n],
                            in_=oo[:, 0:n])

            gB_prev = [None] * NET
            gB_boff = [0] * NET
            layer(0, x_rows_d[0:SPLIT, :], x_rows_d[SPLIT:N, :], p0_s, [0],
                  (idxA_d, idxB_d, lblA, lblB, offA, offB, CAt, CBt))
            gB_prev = [None] * NET
            gB_boff = [0] * NET
            layer(1, h_agL[:, :], h_agH[:, :], p1_s, [ND],
                  (idxA1_d, idxB1_d, lblA1, lblB1, offA1, offB1, CAt1, CBt1))

    nc.compile()
    return nc


_CACHE = {}


def _get_nc(cfg, CAB):
    key = (cfg.N, cfg.E, cfg.n_cores, CAB)
    if key not in _CACHE:
        _CACHE[key] = build_nc(cfg, CAB)
    return _CACHE[key]


def kernel(**inputs):
    cfg = Cfg()
    in_maps, CAB = host_prep(cfg, inputs)
    nc = _get_nc(cfg, CAB)
    res = run_bass_kernel_spmd(nc, in_maps, core_ids=list(range(cfg.n_cores)))
    out = np.concatenate(
        [r["outT"] for r in res.results], axis=1
    ).T.astype(np.float32)
    sim = np.concatenate(
        [r["simT"] for r in res.results], axis=1
    ).T.astype(np.float32)
    return (np.ascontiguousarray(out), np.ascontiguousarray(sim))
